# revision 1
# baseline (speedup 1.0000x reference)
"""GCN (3-layer GraphConv, norm='both') on 8 Trainium2 NeuronCores — v2.

Self-contained: takes FULL inputs, returns FULL output [N, n_classes].

v2 design (vs v1)
-----------------
- Tables stored fp16 (256B rows): gather bytes halved; AllGather halved.
- dma_gather(transpose=True): gathered rows land FEATURE-MAJOR [128f, K*128]
  (one column per edge, dst-slot-major within each K-block).
- Reduction over K via DVE log-tree on fp16 with first fold into an f32
  accumulator (no precision loss beyond the one-time fp16 table quantization).
- Algebraic folding: relu(nd*Y) = nd*relu(Y) (nd>0), so
  z_next = ns*(relu(nd*Y) @ W) = (ns*nd) * (relu(Y) @ W).
  The tree result acc = Y^T [f, dst] feeds the next-layer matmul DIRECTLY as
  lhsT (no PE transpose!), relu is one tensor_scalar, and ns*nd is applied at
  PSUM eviction (per-partition scalar).
- Final layer: acc3 = Y3^T [cls, dst] f32 is DMA'd out as-is; host applies
  nd scale, +b3, transpose, unpermute.
- Table ids are CHUNK-MAJOR: id = (j//7)*7168 + core*896 + (j%7)*128 + p.
  A chunk (7 tiles x 8 cores) is a contiguous 7168-row table range whose
  AllGather input is a contiguous z slice — each boundary's AllGather is
  split into 7 chunk collectives fired as soon as their z tiles exist,
  overlapping the collective with the remaining SpMM compute.
- 4 SWDGE queues (each queue = its own Q7 core pair doing descriptor
  generation); calls rotate queues per (tile, half) for 4-way desc-gen.
"""

import math
import os

import numpy as np

P = 128
NCORES = 8
MAXK = int(os.environ.get("GCN_MAXK", "8"))   # max gather steps per call
NSWQ = int(os.environ.get("GCN_NSWQ", "4"))    # SWDGE queues (round-robin)
CH_T = int(os.environ.get("GCN_CHT", "7"))     # tiles per AllGather chunk


# ----------------------------------------------------------------------------
# Host-side preprocessing
# ----------------------------------------------------------------------------

def _preprocess(features, edge_index, W1, b1, W2, b2, W3, b3):
    n, din = features.shape
    dhid = W2.shape[0]
    ncls = W3.shape[1]
    assert din == P and dhid == P, "kernel assumes 128-wide features"
    assert not (np.any(b1) or np.any(b2)), "nonzero hidden bias unsupported"

    src = np.asarray(edge_index[0], dtype=np.int64)
    dst = np.asarray(edge_index[1], dtype=np.int64)

    deg_out = np.bincount(src, minlength=n).astype(np.float32)
    deg_in = np.bincount(dst, minlength=n).astype(np.float32)
    ns = np.maximum(deg_out, 1.0) ** -0.5
    nd = np.maximum(deg_in, 1.0) ** -0.5

    tpc = math.ceil((n + 1) / (P * NCORES))       # tiles per core (49)
    s0 = tpc * P                                   # slots per core
    ntot = NCORES * s0
    nv = ntot - n
    assert nv >= 1
    HIGH = min(32768, ntot)
    LOW = ntot - HIGH

    # AllGather chunks in tiles-per-core; chosen so chunks 0+1 cover view A
    # ([0, HIGH)) and chunks 1+2 cover view B ([LOW, ntot)) exactly.
    assert LOW % (P * NCORES) == 0 and HIGH % (P * NCORES) == 0
    c0 = LOW // (P * NCORES)
    c1 = HIGH // (P * NCORES) - c0
    chunks = [c0, c1, tpc - c0 - c1]
    nch = len(chunks)
    cs = [0, c0, c0 + c1]                          # chunk start tile
    row_base = [0, LOW, HIGH, ntot]                # chunk start table row

    def chunk_of_j(j):
        for k in range(nch - 1, -1, -1):
            if j >= cs[k]:
                return k
        raise AssertionError(j)

    def base_of(c, j):
        k = chunk_of_j(j)
        return row_base[k] + c * (chunks[k] * P) + (j - cs[k]) * P

    # id -> (core, j, p)
    ids = np.arange(ntot)
    k_of_id = np.searchsorted(np.array(row_base), ids, side="right") - 1
    cn_arr = np.array(chunks)[k_of_id]
    rem = ids - np.array(row_base)[k_of_id]
    c_of_id = rem // (cn_arr * P)
    j_of_id = np.array(cs)[k_of_id] + (rem % (cn_arr * P)) // P
    p_of_id = ids % P

    # --- zone capacities (A-only / flex / B-only by table id)
    zone_lo = np.array([0, LOW, HIGH])
    zone_hi = np.array([LOW, HIGH, ntot])
    zone_size = zone_hi - zone_lo
    resv = np.zeros(3, dtype=np.int64)
    z_of_id0 = 0 if LOW > 0 else 1
    resv[z_of_id0] += 1
    tail_lo = ntot - (nv - 1)
    for z in range(3):
        resv[z] += max(0, min(zone_hi[z], ntot) - max(zone_lo[z], tail_lo))
    real_cap = zone_size - resv
    assert real_cap.sum() == n and (real_cap >= 0).all()

    # --- assign real nodes to zones: degree-sorted, dealt proportionally
    order = np.argsort(-deg_in, kind="stable")
    zone_of_old = np.empty(n, dtype=np.int8)
    cnt = np.zeros(3, dtype=np.int64)
    err = np.zeros(3)
    frac = real_cap / max(n, 1)
    for idx in order:
        err += frac
        z = int(np.argmax(np.where(cnt < real_cap, err, -np.inf)))
        err[z] -= 1.0
        cnt[z] += 1
        zone_of_old[idx] = z

    # --- forcing + provisional balanced split per dst node
    sz = zone_of_old[src]
    forced_a = sz == 0
    forced_b = sz == 2
    flex = sz == 1
    n_a = np.bincount(dst[forced_a], minlength=n)
    n_b = np.bincount(dst[forced_b], minlength=n)
    n_f = np.bincount(dst[flex], minlength=n)
    deg = n_a + n_b + n_f
    x_prov = np.clip((n_b - n_a + n_f + 1) // 2, 0, n_f)
    ka_prov = n_a + x_prov
    kb_prov = n_b + (n_f - x_prov)

    # --- place nodes into tiles; (j, core) deal order keeps the 8 cores'
    #     j-th tiles profile-matched; table base is chunk-major
    queues = []
    for z in range(3):
        ids_z = np.nonzero(zone_of_old == z)[0]
        o = np.lexsort((-deg[ids_z], -kb_prov[ids_z], -ka_prov[ids_z]))
        q = ids_z[o].tolist()
        if z == z_of_id0:
            q = [-1] + q
        extra = int(resv[z] - (1 if z == z_of_id0 else 0))
        q = q + [-1] * extra
        queues.append(q)
    cur = [0, 0, 0]
    old_of_new = np.full(ntot, -1, dtype=np.int64)
    for j in range(tpc):
        zs = [0 if base_of(c, j) < LOW else
              (1 if base_of(c, j) < HIGH else 2) for c in range(NCORES)]
        for z in sorted(set(zs)):
            zcores = [c for c in range(NCORES) if zs[c] == z]
            g = len(zcores)
            block = queues[z][cur[z]:cur[z] + g * P]
            cur[z] += g * P
            # deal round-robin so each core's tile gets an even profile
            for i, c in enumerate(zcores):
                old_of_new[base_of(c, j):base_of(c, j) + P] = block[i::g]
    assert all(cur[z] == len(queues[z]) for z in range(3))
    assert old_of_new[0] == -1 and old_of_new[ntot - 1] == -1
    mask_real = old_of_new >= 0
    new_of_old = np.empty(n, dtype=np.int64)
    new_of_old[old_of_new[mask_real]] = np.nonzero(mask_real)[0]

    s_new = new_of_old[src]
    t_new = new_of_old[dst]
    pad_a = 0
    pad_b = ntot - 1 - LOW

    # --- per-tile budgets (max over all 8 cores at local tile j)
    j_of_dst = j_of_id[t_new]
    maxna_j = np.zeros(tpc, dtype=np.int64)
    maxnb_j = np.zeros(tpc, dtype=np.int64)
    maxdeg_j = np.zeros(tpc, dtype=np.int64)
    np.maximum.at(maxna_j, j_of_dst, n_a[dst])
    np.maximum.at(maxnb_j, j_of_dst, n_b[dst])
    np.maximum.at(maxdeg_j, j_of_dst, deg[dst])
    ka_j = maxna_j
    kb_j = np.maximum(maxnb_j, maxdeg_j - ka_j)

    # --- final per-node split honoring the budgets
    jv = np.zeros(n, dtype=np.int64)
    jv[dst] = j_of_dst
    lo_k = np.maximum(n_a, deg - kb_j[jv])
    hi_k = np.minimum(n_a + n_f, ka_j[jv])
    assert (lo_k <= hi_k).all()
    k_a = np.clip(ka_prov, lo_k, hi_k)
    x_to_a = k_a - n_a

    flex_idx = np.nonzero(flex)[0]
    o = np.argsort(dst[flex_idx], kind="stable")
    fi = flex_idx[o]
    t_sorted = dst[fi]
    grp_sizes = np.bincount(t_sorted, minlength=n)
    grp_start = np.concatenate([[0], np.cumsum(grp_sizes)])[:-1]
    pos_in = np.arange(len(fi)) - grp_start[t_sorted]
    in_a = forced_a.copy()
    in_a[fi[pos_in < x_to_a[t_sorted]]] = True

    # --- ELL buffers: per core, [P, S] int16
    steps_j = ka_j + kb_j
    offa = np.zeros(tpc, dtype=np.int64)
    offa[1:] = np.cumsum(steps_j)[:-1]
    offb = offa + ka_j
    S = int(np.cumsum(steps_j)[-1]) if tpc > 0 else 0

    col_default = np.empty(S, dtype=np.int16)
    for j in range(tpc):
        col_default[offa[j]:offa[j] + ka_j[j]] = pad_a
        col_default[offb[j]:offb[j] + kb_j[j]] = pad_b
    ell = np.broadcast_to(col_default, (NCORES, P, S)).copy()

    def scatter_half(mask, values, off_arr):
        e = np.nonzero(mask)[0]
        t_e = t_new[e]
        o2 = np.argsort(t_e, kind="stable")
        e = e[o2]
        t_e = t_e[o2]
        gsz = np.bincount(t_e, minlength=ntot)
        gst = np.concatenate([[0], np.cumsum(gsz)])[:-1]
        k = np.arange(len(e)) - gst[t_e]
        c_e = c_of_id[t_e]
        p_e = p_of_id[t_e]
        j_e = j_of_id[t_e]
        col = off_arr[j_e] + k
        ell[c_e, p_e, col] = values[e].astype(np.int16)

    scatter_half(in_a, s_new, offa)
    scatter_half(~in_a, s_new - LOW, offb)

    # --- wrap ELL into dma_gather idx layout: flat i = g*128 + p ->
    #     [i % 16, i // 16], replicated to 128 partitions
    call_list = []  # (j, half, col_off_in_idxbuf, K, o_lo_steps)
    idx_cols = 0
    for j in range(tpc):
        o_lo = 0
        for half, kk in ((0, int(ka_j[j])), (1, int(kb_j[j]))):
            for g0 in range(0, kk, MAXK):
                kc = min(MAXK, kk - g0)
                call_list.append((j, half, idx_cols, kc, o_lo + g0))
                idx_cols += kc * 8
            o_lo += kk

    idx_all = np.zeros((NCORES, P, max(idx_cols, 8)), dtype=np.int16)
    for c in range(NCORES):
        for (j, half, coff, K, o_lo) in call_list:
            src_off = (
                offa[j] + o_lo if half == 0 else offb[j] + (o_lo - ka_j[j])
            )
            blk = ell[c, :, src_off:src_off + K]           # [P, K]
            flat = blk.T.reshape(-1)                       # i = g*128 + p
            w16 = flat.reshape(K * 8, 16).T                # [16, K*8]
            idx_all[c, :, coff:coff + K * 8] = np.tile(w16, (8, 1))

    # --- per-core dense inputs (fp16 features / weights, f32 scales)
    feat_new = np.zeros((ntot, din), dtype=np.float32)
    feat_new[mask_real] = np.asarray(features, dtype=np.float32)[
        old_of_new[mask_real]
    ]
    ns_new = np.zeros(ntot, dtype=np.float32)
    ns_new[mask_real] = ns[old_of_new[mask_real]]
    nd_new = np.zeros(ntot, dtype=np.float32)
    nd_new[mask_real] = nd[old_of_new[mask_real]]

    core_ids = np.empty((NCORES, s0), dtype=np.int64)
    for c in range(NCORES):
        base = np.array([base_of(c, j) for j in range(tpc)])
        core_ids[c] = (base[:, None] + np.arange(P)[None, :]).reshape(-1)

    def per_core_scale(vec):
        return [
            np.ascontiguousarray(
                vec[core_ids[c]].reshape(tpc, P).T
            ) for c in range(NCORES)
        ]

    xct = [
        np.ascontiguousarray(feat_new[core_ids[c]].T.astype(np.float16))
        for c in range(NCORES)
    ]
    s0_scale = per_core_scale(ns_new)                 # z1 scale
    s12_scale = per_core_scale(ns_new * nd_new)       # fused relu scale
    s3_scale = per_core_scale(nd_new)                 # final scale

    w3p = np.zeros((dhid, P), dtype=np.float16)
    w3p[:, :ncls] = np.asarray(W3, dtype=np.float32).astype(np.float16)

    meta = dict(
        n=n, din=din, dhid=dhid, ncls=ncls, tpc=tpc, s0=s0, ntot=ntot,
        nch=nch, chunks=chunks, cs=cs, row_base=row_base,
        LOW=LOW, HIGH=HIGH, call_list=call_list, idx_cols=int(max(idx_cols, 8)),
        ka_j=ka_j.tolist(), kb_j=kb_j.tolist(),
        old_of_new=old_of_new, nd=nd, b3=np.asarray(b3, np.float32),
        c_of_id=c_of_id, j_of_id=j_of_id, p_of_id=p_of_id,
    )
    in_maps = []
    for c in range(NCORES):
        in_maps.append({
            "xct": xct[c],
            "w1": np.asarray(W1, np.float32).astype(np.float16),
            "w2": np.asarray(W2, np.float32).astype(np.float16),
            "w3p": w3p,
            "sc0": s0_scale[c],
            "sc12": s12_scale[c],
            "sc3": s3_scale[c],
            "idx": np.ascontiguousarray(idx_all[c]),
        })
    return meta, in_maps


# ----------------------------------------------------------------------------
# Device program
# ----------------------------------------------------------------------------

def _build_program(meta, enable_asserts=False):
    import concourse.bacc as bacc
    import concourse.mybir as mybir
    import concourse.tile as tile
    from concourse.masks import make_identity

    f32 = mybir.dt.float32
    f16 = mybir.dt.float16
    i16 = mybir.dt.int16
    Alu = mybir.AluOpType
    Act = mybir.ActivationFunctionType

    tpc, s0, ntot = meta["tpc"], meta["s0"], meta["ntot"]
    dhid = meta["dhid"]
    nch, chunks, cs = meta["nch"], meta["chunks"], meta["cs"]
    row_base = meta["row_base"]
    LOW, HIGH = meta["LOW"], meta["HIGH"]
    # process chunk-1 tiles first so the chunk-1 collective (needed by BOTH
    # next-layer views) fires ~1/3 into the layer; chunk-2 last.
    tile_order = (
        list(range(cs[1], cs[1] + chunks[1]))
        + list(range(cs[0], cs[0] + chunks[0]))
        + list(range(cs[2], cs[2] + chunks[2]))
    )
    pos_of = {j: t for t, j in enumerate(tile_order)}
    fire_pos = {
        max(pos_of[j] for j in range(cs[k], cs[k] + chunks[k])): k
        for k in range(nch)
    }
    LAG = int(os.environ.get("GCN_LAG", "0"))
    call_list = meta["call_list"]
    ka_j, kb_j = meta["ka_j"], meta["kb_j"]
    steps_j = [a + b for a, b in zip(ka_j, kb_j)]
    max_steps = max(steps_j)

    calls_by_j = [[] for _ in range(tpc)]
    for (j, half, coff, K, o_lo) in call_list:
        calls_by_j[j].append((half, coff, K, o_lo))

    nc = bacc.Bacc(
        "TRN2", target_bir_lowering=False, debug=False,
        enable_asserts=enable_asserts, num_devices=NCORES,
        num_swdge_queues=NSWQ,
    )
    qload = [0.0] * NSWQ

    def next_queue(rows):
        q = min(range(NSWQ), key=lambda i: qload[i])
        qload[q] += rows + 150.0   # ~fixed overhead in row-equivalents
        return q

    xct = nc.dram_tensor("xct", [P, s0], f16, kind="ExternalInput")
    w1 = nc.dram_tensor("w1", [P, dhid], f16, kind="ExternalInput")
    w2 = nc.dram_tensor("w2", [dhid, dhid], f16, kind="ExternalInput")
    w3p = nc.dram_tensor("w3p", [dhid, P], f16, kind="ExternalInput")
    sc0 = nc.dram_tensor("sc0", [P, tpc], f32, kind="ExternalInput")
    sc12 = nc.dram_tensor("sc12", [P, tpc], f32, kind="ExternalInput")
    sc3 = nc.dram_tensor("sc3", [P, tpc], f32, kind="ExternalInput")
    idx = nc.dram_tensor("idx", [P, meta["idx_cols"]], i16, kind="ExternalInput")
    outp = nc.dram_tensor("outp", [s0, P], f32, kind="ExternalOutput")

    rg = [list(range(NCORES))]

    with tile.TileContext(nc) as tc:
        with (
            tc.tile_pool(name="constp", bufs=1) as constp,
            tc.tile_pool(name="gatherp", bufs=12) as gatherp,
            tc.tile_pool(name="accp", bufs=4) as accp,
            tc.tile_pool(name="workp", bufs=4) as workp,
            tc.tile_pool(name="psumtp", bufs=2, space="PSUM") as psumtp,
            tc.tile_pool(name="psumzp", bufs=2, space="PSUM") as psumzp,
            tc.tile_pool(name="dramp", bufs=1, space="DRAM") as dramp,
        ):
            z1 = dramp.tile([s0, dhid], f16)
            z2 = dramp.tile([s0, dhid], f16)
            z3 = dramp.tile([s0, P], f16)
            t1 = dramp.tile([ntot, dhid], f16)
            t2 = dramp.tile([ntot, dhid], f16)
            t3 = dramp.tile([ntot, P], f16)
            # per-chunk AllGather landing pads (Shared DRAM allows only one
            # writer per tensor, so each chunk collective gets its own)
            tch = {}
            for li, (t, d) in enumerate(((t1, dhid), (t2, dhid), (t3, P))):
                pads = []
                for ch in range(nch):
                    pads.append(dramp.tile(
                        [chunks[ch] * NCORES * P, d], f16, addr_space="Shared",
                        name=f"tch{li}_{ch}",
                    ))
                tch[id(t)] = pads

            xct_sb = constp.tile([P, s0], f16)
            nc.sync.dma_start(out=xct_sb[:], in_=xct[:, :])
            w1_sb = constp.tile([P, dhid], f16)
            nc.sync.dma_start(out=w1_sb[:], in_=w1[:, :])
            w2_sb = constp.tile([P, dhid], f16)
            nc.sync.dma_start(out=w2_sb[:], in_=w2[:, :])
            w3_sb = constp.tile([P, P], f16)
            nc.sync.dma_start(out=w3_sb[:], in_=w3p[:, :])
            sc0_sb = constp.tile([P, tpc], f32)
            nc.sync.dma_start(out=sc0_sb[:], in_=sc0[:, :])
            sc12_sb = constp.tile([P, tpc], f32)
            nc.sync.dma_start(out=sc12_sb[:], in_=sc12[:, :])
            sc3_sb = constp.tile([P, tpc], f32)
            nc.sync.dma_start(out=sc3_sb[:], in_=sc3[:, :])
            idx_sb = constp.tile([P, meta["idx_cols"]], i16)
            nc.sync.dma_start(out=idx_sb[:], in_=idx[:, :])
            ident = constp.tile([P, P], f16)
            make_identity(nc, ident[:])

            no_cc = os.environ.get("GCN_NOCC", "0") == "1"

            def chunk_collective(zbuf, tbuf, ch):
                pad = tch[id(tbuf)][ch]
                zlo, zhi = cs[ch] * P, (cs[ch] + chunks[ch]) * P
                if no_cc:
                    cw = chunks[ch] * P
                    for r in range(NCORES):
                        nc.sync.dma_start(
                            out=tbuf[
                                row_base[ch] + r * cw:
                                row_base[ch] + (r + 1) * cw, :
                            ],
                            in_=zbuf[zlo:zhi, :],
                        )
                    return
                nc.gpsimd.collective_compute(
                    "AllGather", Alu.bypass, replica_groups=rg,
                    ins=[zbuf[zlo:zhi, :].opt()],
                    outs=[pad[:, :].opt()],
                )
                nc.sync.dma_start(
                    out=tbuf[row_base[ch]:row_base[ch + 1], :],
                    in_=pad[:, :],
                )

            # ---- phase 0: z1 = ns * (X @ W1), fp16, chunked AllGather -> t1
            for t, j in enumerate(tile_order):
                zp = psumzp.tile([P, dhid], f32, tag="zp")
                nc.tensor.matmul(
                    out=zp[:], lhsT=xct_sb[:, j * P:(j + 1) * P], rhs=w1_sb[:],
                    start=True, stop=True,
                )
                zt = workp.tile([P, dhid], f16, tag="zt")
                nc.scalar.activation(
                    out=zt[:], in_=zp[:], func=Act.Copy,
                    scale=sc0_sb[:, j:j + 1],
                )
                nc.sync.dma_start(out=z1[j * P:(j + 1) * P, :], in_=zt[:])
                if t in fire_pos:
                    chunk_collective(z1, t1, fire_pos[t])

            def spmm_layer(tbl, d_el, scale_sb, wnext_sb, zout, tout):
                """Staggered SpMM: A-half gathers run LAG tiles ahead of the
                B-half + reduce + z production, so next-layer A-gathers (which
                only need table chunks 0+1) fill the boundary bubble while
                chunk 2 is still collected."""
                no_gather = os.environ.get("GCN_NOGATHER", "0") == "1"
                gbs = {}

                def issue_a(j):
                    S = steps_j[j]
                    if no_gather or S == 0:
                        gbs[j] = None
                        return
                    gb = gatherp.tile([P, max_steps, d_el], f16, tag="gb")
                    gbs[j] = gb
                    for (half, coff, K, o_lo) in calls_by_j[j]:
                        if half != 0:
                            continue
                        nc.gpsimd.dma_gather(
                            out_ap=gb[:, o_lo:o_lo + K, :],
                            in_ap=tbl[0:HIGH, :],
                            idxs_ap=idx_sb[:, coff:coff + K * 8],
                            num_idxs=K * P, num_idxs_reg=K * P,
                            elem_size=d_el, single_packet=True,
                            queue_num=next_queue(K * P),
                        )

                def finish(j):
                    S = 0 if no_gather else steps_j[j]
                    gb = gbs.pop(j)
                    if S > 0:
                        for (half, coff, K, o_lo) in calls_by_j[j]:
                            if half != 1:
                                continue
                            nc.gpsimd.dma_gather(
                                out_ap=gb[:, o_lo:o_lo + K, :],
                                in_ap=tbl[LOW:ntot, :],
                                idxs_ap=idx_sb[:, coff:coff + K * 8],
                                num_idxs=K * P, num_idxs_reg=K * P,
                                elem_size=d_el, single_packet=True,
                                queue_num=next_queue(K * P),
                            )
                        W = (S + 1) // 2
                        acc = accp.tile(
                            [P, (max_steps + 1) // 2, d_el], f32, tag="acc",
                        )
                        h = S // 2
                        if h > 0:
                            nc.vector.tensor_tensor(
                                out=acc[:, 0:h, :], in0=gb[:, 0:h, :],
                                in1=gb[:, h:2 * h, :], op=Alu.add,
                            )
                        if S % 2 == 1:
                            nc.vector.tensor_copy(
                                out=acc[:, h:W, :], in_=gb[:, S - 1:S, :],
                            )
                        k = W
                        while k > 1:
                            h2 = k // 2
                            nc.vector.tensor_tensor(
                                out=acc[:, 0:h2, :], in0=acc[:, 0:h2, :],
                                in1=acc[:, k - h2:k, :], op=Alu.add,
                            )
                            k -= h2
                    else:
                        acc = accp.tile([P, 1, d_el], f32, tag="acc0")
                        nc.vector.memset(acc[:], 0.0)

                    if wnext_sb is None:
                        ot = workp.tile([P, d_el], f32, tag="ot")
                        nc.scalar.activation(
                            out=ot[:], in_=acc[:, 0, :], func=Act.Copy,
                            scale=scale_sb[:, j:j + 1],
                        )
                        nc.sync.dma_start(
                            out=outp[j * P:(j + 1) * P, :], in_=ot[:],
                        )
                    else:
                        ht = workp.tile([P, d_el], f16, tag="ht")
                        nc.scalar.activation(
                            out=ht[:], in_=acc[:, 0, :], func=Act.Relu,
                            scale=scale_sb[:, j:j + 1],
                        )
                        tp = psumtp.tile([P, P], f16, tag="tp")
                        nc.tensor.transpose(out=tp[:], in_=ht[:],
                                            identity=ident[:])
                        htT = workp.tile([P, P], f16, tag="htT")
                        nc.scalar.activation(out=htT[:], in_=tp[:],
                                             func=Act.Copy)
                        zp = psumzp.tile([P, P], f32, tag="zp2")
                        nc.tensor.matmul(
                            out=zp[:], lhsT=htT[:], rhs=wnext_sb[:],
                            start=True, stop=True,
                        )
                        zt = workp.tile([P, P], f16, tag="zt2")
                        nc.scalar.activation(out=zt[:], in_=zp[:],
                                             func=Act.Copy)
                        nc.sync.dma_start(
                            out=zout[j * P:(j + 1) * P, :], in_=zt[:],
                        )

                for t in range(len(tile_order) + LAG):
                    if t < len(tile_order):
                        issue_a(tile_order[t])
                    if t >= LAG:
                        tc_ = t - LAG
                        finish(tile_order[tc_])
                        if zout is not None and tc_ in fire_pos:
                            chunk_collective(zout, tout, fire_pos[tc_])

            spmm_layer(t1, dhid, sc12_sb, w2_sb, z2, t2)
            spmm_layer(t2, dhid, sc12_sb, w3_sb, z3, t3)
            spmm_layer(t3, P, sc3_sb, None, None, None)

    nc.compile()
    return nc


# ----------------------------------------------------------------------------
# Entry point
# ----------------------------------------------------------------------------

_CACHE = {}


def _graph_key(edge_index, shapes):
    e = np.asarray(edge_index)
    return (e.shape, hash(e.tobytes()), shapes)


def run(inputs, trace=False, trace_cores=None):
    features = np.asarray(inputs["features"], dtype=np.float32)
    edge_index = np.asarray(inputs["edge_index"])
    W1, b1 = np.asarray(inputs["W1"]), np.asarray(inputs["b1"])
    W2, b2 = np.asarray(inputs["W2"]), np.asarray(inputs["b2"])
    W3, b3 = np.asarray(inputs["W3"]), np.asarray(inputs["b3"])

    meta, in_maps = _preprocess(features, edge_index, W1, b1, W2, b2, W3, b3)
    key = _graph_key(edge_index, (features.shape,))
    if key not in _CACHE:
        _CACHE[key] = _build_program(meta)
    nc = _CACHE[key]

    import concourse.bass_utils as bass_utils

    res = bass_utils.run_bass_kernel_spmd(
        nc, in_maps, core_ids=list(range(NCORES)),
        trace=trace, trace_cores=trace_cores,
    )
    return _assemble(meta, [r["outp"] for r in res.results]), res


def kernel(**inputs):
    return run(inputs)[0]


def _assemble(meta, outs):
    n, ncls = meta["n"], meta["ncls"]
    old_of_new = meta["old_of_new"]
    b3 = meta["b3"]
    c_of_id, j_of_id, p_of_id = (
        meta["c_of_id"], meta["j_of_id"], meta["p_of_id"]
    )
    result = np.empty((n, ncls), dtype=np.float32)
    ids = np.nonzero(old_of_new >= 0)[0]
    old = old_of_new[ids]
    for c in range(NCORES):
        m = c_of_id[ids] == c
        arr = np.asarray(outs[c])                      # [s0, P]
        rows = j_of_id[ids[m]] * P + p_of_id[ids[m]]
        result[old[m]] = arr[rows][:, :ncls]
    result += b3[None, :]
    return result



# revision 2
# speedup vs baseline: 1.9889x; 1.9889x over previous
"""GCN (3-layer GraphConv, norm='both') on 8 Trainium2 NeuronCores — v3.

Self-contained: takes FULL inputs, returns FULL output [N, n_classes].

v3 design (vs v2)
-----------------
The v2 bottleneck was SWDGE descriptor-generation starvation: the DVE
tree-reduce (tensor_tensor) grabs the shared SBUF port pair, locking the
GPSIMD Q7 cores out of descriptor writes, so gathers ran at ~63 GB/s.

v3 moves the segment reduction to the PE array:
- Edges for dst tile j are packed flat into per-(tile, source-chunk)
  segments (no ELL: ~105k instead of ~128k gathered rows/core/layer).
- dma_gather(transpose=False) lands rows slot-major: gb[p, g, :] = row of
  slot g*128+p.
- Per tile, a one-hot matrix sel[slot, dst] (one DVE is_equal op from a
  host-provided dstof table + an iota row) turns the segment-sum into
  G_j accumulating PE matmuls: psum[dst, f] += sel_g^T @ gb_g.
- relu+ns*nd scale at PSUM eviction (per-partition scale), PE transpose,
  next-layer weight matmul — as in v2.
- AllGather lands in per-chunk Shared-DRAM pads; gathers read the pads
  DIRECTLY with chunk-local int16 indices (v2's pad->table copy is gone).
- SPMD requires one static program: per-(tile, chunk) segment capacity is
  the max over the 8 cores (~5% pad rows, pointed at row 0 with
  dstof=-1 so they contribute nothing).
"""

import math
import os

import numpy as np

P = 128
NCORES = 8
NCH = 3                                        # AllGather chunks per layer
MAXK = int(os.environ.get("GCN_MAXK", "16"))   # max groups per gather call
NSWQ = int(os.environ.get("GCN_NSWQ", "4"))    # SWDGE queues (round-robin)
GB_BUFS = int(os.environ.get("GCN_GBBUFS", "12"))


# ----------------------------------------------------------------------------
# Host-side preprocessing
# ----------------------------------------------------------------------------

def _preprocess(features, edge_index, W1, b1, W2, b2, W3, b3):
    n, din = features.shape
    dhid = W2.shape[0]
    ncls = W3.shape[1]
    assert din == P and dhid == P, "kernel assumes 128-wide features"
    assert not (np.any(b1) or np.any(b2)), "nonzero hidden bias unsupported"

    src = np.asarray(edge_index[0], dtype=np.int64)
    dst = np.asarray(edge_index[1], dtype=np.int64)

    deg_out = np.bincount(src, minlength=n).astype(np.float32)
    deg_in = np.bincount(dst, minlength=n).astype(np.float32)
    ns = np.maximum(deg_out, 1.0) ** -0.5
    nd = np.maximum(deg_in, 1.0) ** -0.5

    tpc = math.ceil(n / (P * NCORES))              # tiles per core (49)
    s0 = tpc * P                                   # slots per core
    ntot = NCORES * s0

    # chunk split (in tiles)
    c0 = tpc // NCH
    chunks = [c0] * (NCH - 1) + [tpc - c0 * (NCH - 1)]
    cs = np.concatenate([[0], np.cumsum(chunks)])[:-1]          # start tile
    rows_ch = [chunks[ch] * NCORES * P for ch in range(NCH)]
    assert all(r <= 32767 for r in rows_ch), "chunk rows must fit int16"
    chunk_of_tile = np.searchsorted(cs, np.arange(tpc), side="right") - 1

    # --- node placement: serpentine deal by deg_in desc into (tile, core)
    # bins so per-bin degree sums (and tile profiles across cores) balance
    nbins = tpc * NCORES
    order = np.argsort(-deg_in, kind="stable")
    i = np.arange(n)
    r = i // nbins                                 # slot within bin (= p)
    b = i % nbins
    odd = (r % 2) == 1
    b = np.where(odd, nbins - 1 - b, b)
    j_of_old = np.empty(n, dtype=np.int64)
    c_of_old = np.empty(n, dtype=np.int64)
    p_of_old = np.empty(n, dtype=np.int64)
    j_of_old[order] = b // NCORES
    c_of_old[order] = b % NCORES
    p_of_old[order] = r

    # --- edge mapping
    ce = c_of_old[dst]
    je = j_of_old[dst]
    pe = p_of_old[dst]
    js = j_of_old[src]
    chs = chunk_of_tile[js]
    loc = (c_of_old[src] * np.array(chunks)[chs] * P
           + (js - cs[chs]) * P + p_of_old[src])
    assert loc.max() < 32768

    # sort edges by (core, tile, src-chunk, src-loc)
    okey = np.lexsort((loc, chs, je, ce))
    ce, je, pe, chs, loc = ce[okey], je[okey], pe[okey], chs[okey], loc[okey]

    # per (core, tile, chunk) counts; static capacity = max over cores
    flatkey = (ce * tpc + je) * NCH + chs
    cnt = np.bincount(flatkey, minlength=NCORES * tpc * NCH).reshape(
        NCORES, tpc, NCH)
    cap = cnt.max(axis=0)                          # [tpc, NCH]
    gseg = -(-cap // P)                            # groups per segment
    G_j = gseg.sum(axis=1)                         # [tpc]
    Gmax = int(G_j.max())
    og_j = np.concatenate([[0], np.cumsum(G_j)])   # dstof col offset per tile
    Gtot = int(og_j[-1])

    # segment group offsets within each tile's gb
    seg_og = np.zeros((tpc, NCH), dtype=np.int64)
    seg_og[:, 1:] = np.cumsum(gseg, axis=1)[:, :-1]

    # --- static call list: (j, ch, o_g, coff, num_idxs)
    call_list = []
    coff = 0
    for j in range(tpc):
        for ch in range(NCH):
            m = int(cap[j, ch])
            o = int(seg_og[j, ch])
            g0 = 0
            while g0 * P < m:
                kk = min(MAXK, gseg[j, ch] - g0)
                num = min(kk * P, m - g0 * P)
                cols = -(-num // 16)
                call_list.append((j, ch, o + g0, coff, num))
                coff += cols
                g0 += kk
    idx_cols = max(coff, 8)

    # --- per-core idx + dstof buffers
    edge_start = np.zeros(NCORES * tpc * NCH + 1, dtype=np.int64)
    np.cumsum(cnt.reshape(-1), out=edge_start[1:])

    idx_all = np.zeros((NCORES, P, idx_cols), dtype=np.int16)
    dstof_all = np.full((NCORES, P, Gtot), -1.0, dtype=np.float16)
    for c in range(NCORES):
        for j in range(tpc):
            for ch in range(NCH):
                k = (c * tpc + j) * NCH + ch
                e0, e1 = edge_start[k], edge_start[k + 1]
                m_real = e1 - e0
                m_cap = int(cap[j, ch])
                locs = np.zeros(m_cap, dtype=np.int16)
                locs[:m_real] = loc[e0:e1]
                pds = np.full(m_cap, -1.0, dtype=np.float16)
                pds[:m_real] = pe[e0:e1].astype(np.float16)
                # dstof: slot i -> [p=i%128, g=seg_og+i//128]
                o = int(seg_og[j, ch])
                gs = int(gseg[j, ch])
                pad_g = gs * P - m_cap
                pdsf = np.concatenate(
                    [pds, np.full(pad_g, -1.0, dtype=np.float16)])
                dstof_all[c, :, og_j[j] + o:og_j[j] + o + gs] = (
                    pdsf.reshape(gs, P).T)
        # idx wrapping per call
        for (j, ch, o_g, cof, num) in call_list:
            k = (c * tpc + j) * NCH + ch
            e0, e1 = edge_start[k], edge_start[k + 1]
            o = int(seg_og[j, ch])
            s_lo = (o_g - o) * P                   # slot offset within seg
            seg_locs = np.zeros(int(cap[j, ch]), dtype=np.int16)
            seg_locs[:e1 - e0] = loc[e0:e1]
            flat = seg_locs[s_lo:s_lo + num]
            cols = -(-num // 16)
            fpad = np.zeros(cols * 16, dtype=np.int16)
            fpad[:num] = flat
            w16 = fpad.reshape(cols, 16).T         # [16, cols]
            idx_all[c, :, cof:cof + cols] = np.tile(w16, (8, 1))

    # --- per-core dense inputs
    mask_real = np.zeros(ntot, dtype=bool)
    new_flat = (c_of_old * tpc + j_of_old) * P + p_of_old
    mask_real[new_flat] = True
    old_of_new = np.full(ntot, -1, dtype=np.int64)
    old_of_new[new_flat] = np.arange(n)

    feat_new = np.zeros((ntot, din), dtype=np.float32)
    feat_new[new_flat] = np.asarray(features, dtype=np.float32)
    ns_new = np.zeros(ntot, dtype=np.float32)
    ns_new[new_flat] = ns
    nd_new = np.zeros(ntot, dtype=np.float32)
    nd_new[new_flat] = nd

    def per_core_scale(vec):
        return [np.ascontiguousarray(
            vec[c * s0:(c + 1) * s0].reshape(tpc, P).T) for c in range(NCORES)]

    xct = [np.ascontiguousarray(
        feat_new[c * s0:(c + 1) * s0].T.astype(np.float16))
        for c in range(NCORES)]
    s0_scale = per_core_scale(ns_new)
    s12_scale = per_core_scale(ns_new * nd_new)
    s3_scale = per_core_scale(nd_new)

    w3p = np.zeros((dhid, P), dtype=np.float16)
    w3p[:, :ncls] = np.asarray(W3, dtype=np.float32).astype(np.float16)

    meta = dict(
        n=n, din=din, dhid=dhid, ncls=ncls, tpc=tpc, s0=s0, ntot=ntot,
        chunks=chunks, cs=cs.tolist(), rows_ch=rows_ch,
        call_list=call_list, idx_cols=idx_cols,
        G_j=G_j.tolist(), og_j=og_j.tolist(), Gmax=Gmax, Gtot=Gtot,
        slots=int(cap.sum()),
        old_of_new=old_of_new, b3=np.asarray(b3, np.float32),
    )
    in_maps = []
    for c in range(NCORES):
        in_maps.append({
            "xct": xct[c],
            "w1": np.asarray(W1, np.float32).astype(np.float16),
            "w2": np.asarray(W2, np.float32).astype(np.float16),
            "w3p": w3p,
            "sc0": s0_scale[c],
            "sc12": s12_scale[c],
            "sc3": s3_scale[c],
            "idx": np.ascontiguousarray(idx_all[c]),
            "dstof": np.ascontiguousarray(dstof_all[c]),
        })
    return meta, in_maps


# ----------------------------------------------------------------------------
# Device program
# ----------------------------------------------------------------------------

def _build_program(meta, enable_asserts=False):
    import concourse.bacc as bacc
    import concourse.mybir as mybir
    import concourse.tile as tile
    from concourse.masks import make_identity

    f32 = mybir.dt.float32
    f16 = mybir.dt.float16
    i16 = mybir.dt.int16
    Alu = mybir.AluOpType
    Act = mybir.ActivationFunctionType

    tpc, s0 = meta["tpc"], meta["s0"]
    dhid = meta["dhid"]
    chunks, cs, rows_ch = meta["chunks"], meta["cs"], meta["rows_ch"]
    call_list = meta["call_list"]
    G_j, og_j, Gmax = meta["G_j"], meta["og_j"], meta["Gmax"]
    idx_cols, Gtot = meta["idx_cols"], meta["Gtot"]

    calls_by_j = [[] for _ in range(tpc)]
    for (j, ch, o_g, coff, num) in call_list:
        calls_by_j[j].append((ch, o_g, coff, num))

    fire_pos = {cs[ch] + chunks[ch] - 1: ch for ch in range(NCH)}

    nc = bacc.Bacc(
        "TRN2", target_bir_lowering=False, debug=False,
        enable_asserts=enable_asserts, num_devices=NCORES,
        num_swdge_queues=NSWQ,
    )
    qload = [0.0] * NSWQ

    def next_queue(rows):
        q = min(range(NSWQ), key=lambda i: qload[i])
        qload[q] += rows + 150.0
        return q

    xct = nc.dram_tensor("xct", [P, s0], f16, kind="ExternalInput")
    w1 = nc.dram_tensor("w1", [P, dhid], f16, kind="ExternalInput")
    w2 = nc.dram_tensor("w2", [dhid, dhid], f16, kind="ExternalInput")
    w3p = nc.dram_tensor("w3p", [dhid, P], f16, kind="ExternalInput")
    sc0 = nc.dram_tensor("sc0", [P, tpc], f32, kind="ExternalInput")
    sc12 = nc.dram_tensor("sc12", [P, tpc], f32, kind="ExternalInput")
    sc3 = nc.dram_tensor("sc3", [P, tpc], f32, kind="ExternalInput")
    idx = nc.dram_tensor("idx", [P, idx_cols], i16, kind="ExternalInput")
    dstof = nc.dram_tensor("dstof", [P, Gtot], f16, kind="ExternalInput")
    outp = nc.dram_tensor("outp", [s0, P], f32, kind="ExternalOutput")

    rg = [list(range(NCORES))]

    with tile.TileContext(nc) as tc:
        with (
            tc.tile_pool(name="constp", bufs=1) as constp,
            tc.tile_pool(name="gatherp", bufs=GB_BUFS) as gatherp,
            tc.tile_pool(name="selp", bufs=3) as selp,
            tc.tile_pool(name="workp", bufs=4) as workp,
            tc.tile_pool(name="psumap", bufs=2, space="PSUM") as psumap,
            tc.tile_pool(name="psumtp", bufs=2, space="PSUM") as psumtp,
            tc.tile_pool(name="psumzp", bufs=2, space="PSUM") as psumzp,
            tc.tile_pool(name="dramp", bufs=1, space="DRAM") as dramp,
        ):
            z1 = dramp.tile([s0, dhid], f16)
            z2 = dramp.tile([s0, dhid], f16)
            z3 = dramp.tile([s0, P], f16)
            # per-(layer, chunk) AllGather landing pads; gathers read these
            # directly with chunk-local indices
            pads = []
            for li, d in ((0, dhid), (1, dhid), (2, P)):
                pads.append([dramp.tile(
                    [rows_ch[ch], d], f16, addr_space="Shared",
                    name=f"pad{li}_{ch}") for ch in range(NCH)])

            xct_sb = constp.tile([P, s0], f16)
            nc.sync.dma_start(out=xct_sb[:], in_=xct[:, :])
            w1_sb = constp.tile([P, dhid], f16)
            nc.sync.dma_start(out=w1_sb[:], in_=w1[:, :])
            w2_sb = constp.tile([P, dhid], f16)
            nc.sync.dma_start(out=w2_sb[:], in_=w2[:, :])
            w3_sb = constp.tile([P, P], f16)
            nc.sync.dma_start(out=w3_sb[:], in_=w3p[:, :])
            sc0_sb = constp.tile([P, tpc], f32)
            nc.sync.dma_start(out=sc0_sb[:], in_=sc0[:, :])
            sc12_sb = constp.tile([P, tpc], f32)
            nc.sync.dma_start(out=sc12_sb[:], in_=sc12[:, :])
            sc3_sb = constp.tile([P, tpc], f32)
            nc.sync.dma_start(out=sc3_sb[:], in_=sc3[:, :])
            idx_sb = constp.tile([P, idx_cols], i16)
            nc.sync.dma_start(out=idx_sb[:], in_=idx[:, :])
            dst_sb = constp.tile([P, Gtot], f16)
            nc.sync.dma_start(out=dst_sb[:], in_=dstof[:, :])
            ident = constp.tile([P, P], f16)
            make_identity(nc, ident[:])
            iota_i = constp.tile([P, P], i16)
            nc.gpsimd.iota(iota_i[:], pattern=[[1, P]], channel_multiplier=0)
            iota_sb = constp.tile([P, P], f16)
            nc.vector.tensor_copy(out=iota_sb[:], in_=iota_i[:])

            # pre-zero the gather bufs once so unwritten tail slots read 0.0
            # forever after (sel is 0 there, but stale SBUF could be NaN)
            for _ in range(GB_BUFS):
                gz = gatherp.tile([P, Gmax, P], f16, tag="gb")
                nc.vector.memset(gz[:], 0.0)

            def chunk_collective(zbuf, li, ch):
                zlo, zhi = cs[ch] * P, (cs[ch] + chunks[ch]) * P
                nc.gpsimd.collective_compute(
                    "AllGather", Alu.bypass, replica_groups=rg,
                    ins=[zbuf[zlo:zhi, :].opt()],
                    outs=[pads[li][ch][:, :].opt()],
                )

            # ---- phase 0: z1 = ns * (X @ W1) -> chunked AllGather -> pads[0]
            for j in range(tpc):
                zp = psumzp.tile([P, dhid], f32, tag="zp")
                nc.tensor.matmul(
                    out=zp[:], lhsT=xct_sb[:, j * P:(j + 1) * P], rhs=w1_sb[:],
                    start=True, stop=True,
                )
                zt = workp.tile([P, dhid], f16, tag="zt")
                nc.scalar.activation(
                    out=zt[:], in_=zp[:], func=Act.Copy,
                    scale=sc0_sb[:, j:j + 1],
                )
                nc.sync.dma_start(out=z1[j * P:(j + 1) * P, :], in_=zt[:])
                if j in fire_pos:
                    chunk_collective(z1, 0, fire_pos[j])

            def spmm_layer(li, d_el, scale_sb, wnext_sb, zout, li_out):
                for j in range(tpc):
                    Gj = G_j[j]
                    gb = gatherp.tile([P, Gmax, P], f16, tag="gb")
                    for (ch, o_g, coff, num) in calls_by_j[j]:
                        K = -(-num // P)
                        nc.gpsimd.dma_gather(
                            out_ap=gb[:, o_g:o_g + K, :],
                            in_ap=pads[li][ch][:, :],
                            idxs_ap=idx_sb[:, coff:coff + (-(-num // 16))],
                            num_idxs=num, num_idxs_reg=num,
                            elem_size=d_el, single_packet=True,
                            queue_num=next_queue(num),
                        )
                    sel = selp.tile([P, Gmax, P], f16, tag="sel")
                    og = og_j[j]
                    nc.vector.tensor_tensor(
                        out=sel[:, 0:Gj, :],
                        in0=dst_sb[:, og:og + Gj, None].broadcast_to(
                            [P, Gj, P]),
                        in1=iota_sb[:, None, :].broadcast_to([P, Gj, P]),
                        op=Alu.is_equal,
                    )
                    acc = psumap.tile([P, P], f32, tag="acc")
                    for g in range(Gj):
                        nc.tensor.matmul(
                            out=acc[:], lhsT=sel[:, g, :], rhs=gb[:, g, :],
                            start=(g == 0), stop=(g == Gj - 1),
                        )
                    if wnext_sb is None:
                        ot = workp.tile([P, P], f32, tag="ot")
                        nc.scalar.activation(
                            out=ot[:], in_=acc[:], func=Act.Copy,
                            scale=scale_sb[:, j:j + 1],
                        )
                        nc.sync.dma_start(
                            out=outp[j * P:(j + 1) * P, :], in_=ot[:])
                    else:
                        ht = workp.tile([P, P], f16, tag="ht")
                        nc.scalar.activation(
                            out=ht[:], in_=acc[:], func=Act.Relu,
                            scale=scale_sb[:, j:j + 1],
                        )
                        tp = psumtp.tile([P, P], f16, tag="tp")
                        nc.tensor.transpose(out=tp[:], in_=ht[:],
                                            identity=ident[:])
                        htT = workp.tile([P, P], f16, tag="htT")
                        nc.scalar.activation(out=htT[:], in_=tp[:],
                                             func=Act.Copy)
                        zp = psumzp.tile([P, P], f32, tag="zp2")
                        nc.tensor.matmul(
                            out=zp[:], lhsT=htT[:], rhs=wnext_sb[:],
                            start=True, stop=True,
                        )
                        zt = workp.tile([P, P], f16, tag="zt2")
                        nc.scalar.activation(out=zt[:], in_=zp[:],
                                             func=Act.Copy)
                        nc.sync.dma_start(
                            out=zout[j * P:(j + 1) * P, :], in_=zt[:])
                        if j in fire_pos:
                            chunk_collective(zout, li_out, fire_pos[j])

            spmm_layer(0, dhid, sc12_sb, w2_sb, z2, 1)
            spmm_layer(1, dhid, sc12_sb, w3_sb, z3, 2)
            spmm_layer(2, P, sc3_sb, None, None, None)

    nc.compile()
    return nc


# ----------------------------------------------------------------------------
# Entry point
# ----------------------------------------------------------------------------

_CACHE = {}


def _graph_key(edge_index, shapes):
    e = np.asarray(edge_index)
    return (e.shape, hash(e.tobytes()), shapes)


def run(inputs, trace=False, trace_cores=None):
    features = np.asarray(inputs["features"], dtype=np.float32)
    edge_index = np.asarray(inputs["edge_index"])
    W1, b1 = np.asarray(inputs["W1"]), np.asarray(inputs["b1"])
    W2, b2 = np.asarray(inputs["W2"]), np.asarray(inputs["b2"])
    W3, b3 = np.asarray(inputs["W3"]), np.asarray(inputs["b3"])

    meta, in_maps = _preprocess(features, edge_index, W1, b1, W2, b2, W3, b3)
    key = _graph_key(edge_index, (features.shape,))
    if key not in _CACHE:
        _CACHE[key] = _build_program(meta)
    nc = _CACHE[key]

    import concourse.bass_utils as bass_utils

    res = bass_utils.run_bass_kernel_spmd(
        nc, in_maps, core_ids=list(range(NCORES)),
        trace=trace, trace_cores=trace_cores,
    )
    return _assemble(meta, [r["outp"] for r in res.results]), res


def kernel(**inputs):
    return run(inputs)[0]


def _assemble(meta, outs):
    n, ncls, tpc, s0 = meta["n"], meta["ncls"], meta["tpc"], meta["s0"]
    old_of_new = meta["old_of_new"]
    b3 = meta["b3"]
    result = np.empty((n, ncls), dtype=np.float32)
    for c in range(NCORES):
        ids = old_of_new[c * s0:(c + 1) * s0]
        m = ids >= 0
        arr = np.asarray(outs[c])                  # [s0, P]
        result[ids[m]] = arr[m][:, :ncls]
    result += b3[None, :]
    return result
